# revision 15
# baseline (speedup 1.0000x reference)
"""Multi-head GAT layer on 8 Trainium2 NeuronCores (Bass/Tile).

Strategy: nodes are sharded 6250/core (8 cores). Each core handles all edges
whose dst lands in its shard. Edges are grouped into 49 "windows" of 128
dst-slots (host-balanced by degree).

v3 design:
- node table (wv | el | er) stored bf16 in DRAM as [N, 512] rows (1024B,
  256B-aligned for dma_gather).
- P1 builds the table with 4-tile batched DMAs (loads on Sync, stores on
  Scalar HWDGE queue) and bf16 matmuls.
- P2 fetches wv|el for all edges of a window with TWO dma_gather
  instructions (int16 indices, table split in halves at row S) instead of
  17 per-column indirect DMAs. Per-edge er[dst] is computed on the tensor
  engine: one tiny indirect gather of the window's own er rows plus
  host-uploaded transposed one-hot matrices. Edge softmax weights + one-hot
  scatter matmul in bf16; normalize, project, residual-add per window.
- LayerNorm is split: per-window mean/var (DVE only), a single Sqrt at the
  end (avoids Exp<->Sqrt activation-table thrashing), one big output store.
"""
import sys, os, types, ctypes, contextlib

if '/opt/trn_rl_repo' not in sys.path:
    sys.path.insert(0, '/opt/trn_rl_repo')


def _install_profile_hook():
    try:
        import antenv.axon_hooks  # noqa
        return
    except ImportError:
        pass
    try:
        import antenv
    except ImportError:
        return
    so_path = "/opt/axon/libaxon_pjrt.so"
    hook = None
    if os.path.exists(so_path):
        lib = ctypes.CDLL(so_path)
        if hasattr(lib, "axon_start_nrt_profile"):
            lib.axon_start_nrt_profile.argtypes = [ctypes.POINTER(ctypes.c_int64), ctypes.c_size_t]
            lib.axon_start_nrt_profile.restype = ctypes.c_int64
            lib.axon_stop_nrt_profile.argtypes = [ctypes.c_char_p]
            lib.axon_stop_nrt_profile.restype = ctypes.c_int64

            @contextlib.contextmanager
            def _hook(output_dir, device_ids):
                import jax
                jax.devices()
                if device_ids:
                    ids = (ctypes.c_int64 * len(device_ids))(*device_ids)
                    rc = lib.axon_start_nrt_profile(ids, len(device_ids))
                else:
                    rc = lib.axon_start_nrt_profile(None, 0)
                if rc != 0:
                    raise RuntimeError(f"axon_start_nrt_profile rc={rc}")
                try:
                    yield
                finally:
                    n = lib.axon_stop_nrt_profile(str(output_dir).encode())
                    print(f"ntff profile: {n} file(s) -> {output_dir}", file=sys.stderr)
            hook = _hook
    mod = types.ModuleType("antenv.axon_hooks")
    state = {"hook": hook}
    mod.set_axon_ntff_profile_hook = lambda h: state.__setitem__("hook", h)
    mod.get_axon_ntff_profile_hook = lambda: state["hook"]
    sys.modules["antenv.axon_hooks"] = mod
    antenv.axon_hooks = mod


_install_profile_hook()

import numpy as np
import ml_dtypes
from concourse import bass, bacc, mybir, tile
from concourse.bass_utils import run_bass_kernel_spmd

N_NODES = 50000
F = 128
H = 3
NCORES = 8
NPC = N_NODES // NCORES          # 6250 nodes per core
NWIN = (NPC + 127) // 128        # 49 windows per core
NEG_SLOPE = 0.2
LN_EPS = 1e-5
ROWW = 512                        # table row: wv 384 | el 3 | er 3 | pad
SPLIT = 25088                     # table half split (int16 index range)

f32 = mybir.dt.float32
bf16 = mybir.dt.bfloat16
i32 = mybir.dt.int32
i16 = mybir.dt.int16

_PROGRAM_CACHE = {}


def _build_program(GA, GB, zb, gb1):
    T2 = GA + GB
    nc = bacc.Bacc("TRN2", target_bir_lowering=False, debug=False,
                   enable_asserts=False, num_devices=NCORES,
                   dynamic_dma_scratch_size=65536)
    NT = (N_NODES + 127) // 128   # 391 table tiles (last partial)

    hTb_in = nc.dram_tensor("hTb", [F, N_NODES], bf16, kind="ExternalInput").ap()
    wcomb_in = nc.dram_tensor("wcomb", [128, 390], bf16, kind="ExternalInput").ap()
    brow_in = nc.dram_tensor("brow", [128, 390], f32, kind="ExternalInput").ap()
    iotaw_in = nc.dram_tensor("iotaw", [128, T2 * 128], bf16, kind="ExternalInput").ap()
    ident_in = nc.dram_tensor("ident", [128, 128], bf16, kind="ExternalInput").ap()
    wsc_in = nc.dram_tensor("wsc", [H * F, F], bf16, kind="ExternalInput").ap()
    gam_in = nc.dram_tensor("gam", [128, F], f32, kind="ExternalInput").ap()
    bet_in = nc.dram_tensor("bet", [128, F], f32, kind="ExternalInput").ap()
    hperm_in = nc.dram_tensor("hperm", [NWIN * 128, F], f32, kind="ExternalInput").ap()
    widx_in = nc.dram_tensor("widx", [NWIN, 128, T2 * 8], i16, kind="ExternalInput").ap()
    wnid_in = nc.dram_tensor("wnid", [NWIN, 128, 1], i32, kind="ExternalInput").ap()
    wdb_in = nc.dram_tensor("wdb", [NWIN, 128, T2], bf16, kind="ExternalInput").ap()
    ohtb_in = nc.dram_tensor("ohtb", [NWIN, 128, T2 * 128], bf16, kind="ExternalInput").ap()

    table = nc.dram_tensor("table", [N_NODES, ROWW], bf16).ap()
    outy = nc.dram_tensor("outy", [NWIN * 128, F], f32, kind="ExternalOutput").ap()

    inv_f = 1.0 / F

    with tile.TileContext(nc) as tc:
        with tc.tile_pool(name="const", bufs=1) as cpool:
            wcomb = cpool.tile([128, 390], bf16)
            nc.sync.dma_start(wcomb[:], wcomb_in[:])
            brow = cpool.tile([128, 390], f32)
            nc.sync.dma_start(brow[:], brow_in[:])
            iotaw = cpool.tile([128, T2 * 128], bf16)
            nc.sync.dma_start(iotaw[:], iotaw_in[:])
            ident = cpool.tile([128, 128], bf16)
            nc.sync.dma_start(ident[:], ident_in[:])
            gam = cpool.tile([128, F], f32)
            nc.sync.dma_start(gam[:], gam_in[:])
            bet = cpool.tile([128, F], f32)
            nc.sync.dma_start(bet[:], bet_in[:])
            wsc_c = []
            for c in range(H):
                t = cpool.tile([128, F], bf16, tag=f"wsc{c}")
                nc.sync.dma_start(t[:], wsc_in[c * 128:(c + 1) * 128, :])
                wsc_c.append(t)
            # persistent LN state
            xc_all = cpool.tile([128, NWIN * F], f32, tag="xc_all")
            vp_all = cpool.tile([128, NWIN], f32, tag="vp_all")

            # ---- P1: build wv/el/er table (4-tile batches)
            with (
                tc.tile_pool(name="p1l", bufs=3) as lpool,
                tc.tile_pool(name="p1d", bufs=3) as dpool,
                tc.tile_pool(name="p1p", bufs=2, space="PSUM") as p1ps,
            ):
                NB = 97  # full 4-tile batches
                for b in range(NB):
                    n0 = b * 512
                    ht4 = lpool.tile([128, 512], bf16, tag="ht")
                    nc.sync.dma_start(ht4[:], hTb_in[:, n0:n0 + 512])
                    sb4 = dpool.tile([128, 4, 390], bf16, tag="sb")
                    for t in range(4):
                        ps = p1ps.tile([128, 390], f32, tag=f"p{t}")
                        nc.tensor.matmul(out=ps[:], lhsT=ht4[:, t * 128:(t + 1) * 128],
                                         rhs=wcomb[:], start=True, stop=True)
                        if zb:
                            if t % 2 == 0:
                                nc.vector.tensor_copy(sb4[:, t, :], ps[:])
                            else:
                                nc.scalar.copy(sb4[:, t, :], ps[:])
                        else:
                            nc.vector.tensor_tensor(out=sb4[:, t, :], in0=ps[:], in1=brow[:],
                                                    op=mybir.AluOpType.add)
                    nc.scalar.dma_start(
                        table[n0:n0 + 512, 0:390].rearrange("(t p) c -> p t c", p=128),
                        sb4[:])
                for tt in range(4 * NB, NT):
                    n0 = tt * 128
                    n1 = min(n0 + 128, N_NODES)
                    p = n1 - n0
                    ht1 = lpool.tile([128, 128], bf16, tag="ht1")
                    nc.sync.dma_start(ht1[:, :p], hTb_in[:, n0:n1])
                    ps = p1ps.tile([128, 390], f32, tag="p0")
                    nc.tensor.matmul(out=ps[:p, :], lhsT=ht1[:, :p], rhs=wcomb[:],
                                     start=True, stop=True)
                    sb1 = dpool.tile([128, 390], bf16, tag="sb1")
                    if zb:
                        nc.vector.tensor_copy(sb1[:p, :], ps[:p, :])
                    else:
                        nc.vector.tensor_tensor(out=sb1[:p, :], in0=ps[:p, :], in1=brow[:p, :],
                                                op=mybir.AluOpType.add)
                    nc.scalar.dma_start(table[n0:n1, 0:390], sb1[:p, :])

            tabA = table[0:SPLIT, :]
            tabB = table[SPLIT:N_NODES, :]
            # [N*4, 128] view: the er triple sits at subrow 4n+3, cols 0:3
            tab_sub = table[:].rearrange("n (a b) -> (n a) b", b=128)

            # ---- P2: edge windows
            with (
                tc.tile_pool(name="gath", bufs=3) as gpool,
                tc.tile_pool(name="ohp", bufs=2) as opool,
                tc.tile_pool(name="ohtp", bufs=2) as otpool,
                tc.tile_pool(name="small", bufs=3) as spool,
                tc.tile_pool(name="edge", bufs=3) as epool,
                tc.tile_pool(name="fin", bufs=3) as fpool,
                tc.tile_pool(name="psw", bufs=2, space="PSUM") as pswin,
                tc.tile_pool(name="pse", bufs=2, space="PSUM") as pserr,
                tc.tile_pool(name="pst", bufs=2, space="PSUM") as pstab,
                tc.tile_pool(name="psx", bufs=2, space="PSUM") as psaux,
            ):
                for w in range(NWIN):
                    widx = spool.tile([128, T2 * 8], i16, tag="wx")
                    nc.sync.dma_start(widx[:], widx_in[w])
                    wdv = spool.tile([128, T2], bf16, tag="wd")
                    nc.sync.dma_start(wdv[:], wdb_in[w])
                    wnid = spool.tile([128, 1], i32, tag="wn")
                    nc.sync.dma_start(wnid[:], wnid_in[w])
                    # own-node er rows first (small; unblocks the er matmul chain)
                    erw = spool.tile([128, 3], bf16, tag="erw")
                    nc.gpsimd.indirect_dma_start(
                        out=erw[:], out_offset=None, in_=tab_sub,
                        in_offset=bass.IndirectOffsetOnAxis(ap=wnid[:], axis=0))
                    oht = otpool.tile([128, T2 * 128], bf16, tag="oht")
                    nc.sync.dma_start(oht[:], ohtb_in[w])
                    hpw = spool.tile([128, F], f32, tag="hp")
                    nc.scalar.dma_start(hpw[:], hperm_in[w * 128:(w + 1) * 128, :])

                    gv = gpool.tile([128, T2, ROWW], bf16, tag="gv")
                    # dma_gather is limited to 1024 indices (8 groups) per call
                    for g0 in range(0, GA, 8):
                        g1 = min(g0 + 8, GA)
                        nc.gpsimd.dma_gather(gv[:, g0:g1, :], tabA,
                                             widx[:, g0 * 8:g1 * 8],
                                             (g1 - g0) * 128, (g1 - g0) * 128, ROWW)
                    for g0 in range(0, GB, 8):
                        g1 = min(g0 + 8, GB)
                        nc.gpsimd.dma_gather(gv[:, GA + g0:GA + g1, :], tabB,
                                             widx[:, (GA + g0) * 8:(GA + g1) * 8],
                                             (g1 - g0) * 128, (g1 - g0) * 128, ROWW)
                    # per-edge er[dst] via transposed one-hots
                    erp = pserr.tile([128, T2 * 3], f32, tag="erp")
                    for j in range(T2):
                        nc.tensor.matmul(out=erp[:, j * 3:(j + 1) * 3],
                                         lhsT=oht[:, j * 128:(j + 1) * 128],
                                         rhs=erw[:], start=True, stop=True)
                    erb = epool.tile([128, T2 * 3], bf16, tag="erb")
                    nc.vector.tensor_copy(erb[:], erp[:])

                    nk = T2 * 3
                    attn = epool.tile([128, T2, 3], f32, tag="attn")
                    nc.vector.tensor_tensor(out=attn[:], in0=gv[:, :, 387:390],
                                            in1=erb[:].rearrange("p (a b) -> p a b", b=3),
                                            op=mybir.AluOpType.add)
                    at2 = epool.tile([128, nk], f32, tag="at2")
                    nc.vector.tensor_scalar_mul(at2[:], attn[:].rearrange("p a b -> p (a b)"),
                                                NEG_SLOPE)
                    at3 = epool.tile([128, nk], f32, tag="at3")
                    nc.vector.tensor_tensor(out=at3[:], in0=attn[:].rearrange("p a b -> p (a b)"),
                                            in1=at2[:], op=mybir.AluOpType.max)
                    ew = epool.tile([128, T2, 3], bf16, tag="ew")
                    nc.scalar.activation(ew[:].rearrange("p a b -> p (a b)"), at3[:],
                                         mybir.ActivationFunctionType.Exp)

                    oh = opool.tile([128, T2, 128], bf16, tag="oh")
                    nc.vector.tensor_tensor(
                        out=oh[:],
                        in0=wdv[:].rearrange("p (j o) -> p j o", o=1).to_broadcast([128, T2, 128]),
                        in1=iotaw[:].rearrange("p (j f) -> p j f", f=128),
                        op=mybir.AluOpType.is_equal)

                    # scale gathered wv rows by edge weights in place
                    for c in range(H):
                        nc.vector.tensor_tensor(
                            out=gv[:, :, c * 128:(c + 1) * 128],
                            in0=gv[:, :, c * 128:(c + 1) * 128],
                            in1=ew[:, :, c:c + 1].to_broadcast([128, T2, 128]),
                            op=mybir.AluOpType.mult)

                    ps = pswin.tile([128, 387], f32, tag="win")
                    for j in range(T2):
                        nc.tensor.matmul(out=ps[:, 0:384], lhsT=oh[:, j, :],
                                         rhs=gv[:, j, 0:384],
                                         start=(j == 0), stop=(j == T2 - 1))
                        nc.tensor.matmul(out=ps[:, 384:387], lhsT=oh[:, j, :],
                                         rhs=ew[:, j, :],
                                         start=(j == 0), stop=(j == T2 - 1))

                    dmx = spool.tile([128, 3], f32, tag="dmx")
                    nc.vector.tensor_scalar(out=dmx[:], in0=ps[:, 384:387], scalar1=1e-9,
                                            scalar2=None, op0=mybir.AluOpType.max)
                    dr = spool.tile([128, 3], f32, tag="dr")
                    nc.vector.reciprocal(dr[:], dmx[:])
                    agg = epool.tile([128, 3, 128], bf16, tag="agg")
                    nc.vector.tensor_tensor(
                        out=agg[:],
                        in0=ps[:, 0:384].rearrange("p (c f) -> p c f", f=128),
                        in1=dr[:].rearrange("p (c o) -> p c o", o=1).to_broadcast([128, 3, 128]),
                        op=mybir.AluOpType.mult)

                    pp = psaux.tile([128, F], f32, tag="proj")
                    for c in range(H):
                        tp = pstab.tile([128, 128], bf16, tag="tp")
                        nc.tensor.transpose(out=tp[:], in_=agg[:, c, :], identity=ident[:])
                        aggT = epool.tile([128, 128], bf16, tag="aggT")
                        nc.vector.tensor_copy(aggT[:], tp[:])
                        nc.tensor.matmul(out=pp[:], lhsT=aggT[:], rhs=wsc_c[c][:],
                                         start=(c == 0), stop=(c == H - 1))

                    # residual + LN stats (mean/var); normalization deferred
                    xw = fpool.tile([128, F], f32, tag="xw")
                    sm = fpool.tile([128, 1], f32, tag="sm")
                    nc.vector.tensor_tensor(out=xw[:], in0=pp[:], in1=hpw[:],
                                            op=mybir.AluOpType.add)
                    nc.vector.reduce_sum(sm[:], xw[:], axis=mybir.AxisListType.X)
                    mu = fpool.tile([128, 1], f32, tag="mu")
                    nc.vector.tensor_scalar_mul(mu[:], sm[:], inv_f)
                    nc.vector.tensor_scalar(out=xc_all[:, w * F:(w + 1) * F], in0=xw[:],
                                            scalar1=mu[:, 0:1], scalar2=None,
                                            op0=mybir.AluOpType.subtract)
                    junk = fpool.tile([128, F], f32, tag="junk")
                    nc.vector.tensor_tensor(out=junk[:], in0=xc_all[:, w * F:(w + 1) * F],
                                            in1=xc_all[:, w * F:(w + 1) * F],
                                            op=mybir.AluOpType.mult)
                    vs = fpool.tile([128, 1], f32, tag="vs")
                    nc.vector.reduce_sum(vs[:], junk[:], axis=mybir.AxisListType.X)
                    nc.vector.tensor_scalar_mul(vp_all[:, w:w + 1], vs[:], inv_f)

                # ---- P3: single sqrt, normalize, relu, one big store
                vpe = cpool.tile([128, NWIN], f32, tag="vpe")
                nc.vector.tensor_scalar(out=vpe[:], in0=vp_all[:], scalar1=LN_EPS,
                                        scalar2=None, op0=mybir.AluOpType.add)
                rv = cpool.tile([128, NWIN], f32, tag="rv")
                nc.vector.reciprocal(rv[:], vpe[:])
                rstd = cpool.tile([128, NWIN], f32, tag="rstd")
                nc.scalar.activation(rstd[:], rv[:], mybir.ActivationFunctionType.Sqrt)
                for w in range(NWIN):
                    y1 = fpool.tile([128, F], f32, tag="y1")
                    if gb1:
                        # fuse: y = max(xc * rstd, 0)
                        nc.vector.tensor_scalar(out=xc_all[:, w * F:(w + 1) * F],
                                                in0=xc_all[:, w * F:(w + 1) * F],
                                                scalar1=rstd[:, w:w + 1], scalar2=0.0,
                                                op0=mybir.AluOpType.mult,
                                                op1=mybir.AluOpType.max)
                    else:
                        nc.vector.tensor_scalar_mul(y1[:], xc_all[:, w * F:(w + 1) * F],
                                                    rstd[:, w:w + 1])
                        y2 = fpool.tile([128, F], f32, tag="y2")
                        nc.vector.tensor_tensor(out=y2[:], in0=y1[:], in1=gam[:],
                                                op=mybir.AluOpType.mult)
                        y3 = fpool.tile([128, F], f32, tag="y3")
                        nc.vector.tensor_tensor(out=y3[:], in0=y2[:], in1=bet[:],
                                                op=mybir.AluOpType.add)
                        nc.vector.tensor_scalar(out=xc_all[:, w * F:(w + 1) * F], in0=y3[:],
                                                scalar1=0.0, scalar2=None,
                                                op0=mybir.AluOpType.max)
                nc.scalar.dma_start(outy.rearrange("(w p) f -> p w f", p=128),
                                    xc_all[:].rearrange("p (w f) -> p w f", f=F))

    nc.compile()
    return nc


def _wrap_idx(ix, cap):
    """Pad index list to cap and wrap for dma_gather: idx i -> [p, s] with
    i = s*16 + p%16, replicated across the 8 16-partition groups."""
    arr = np.zeros(cap, np.int16)
    arr[:len(ix)] = ix
    return np.tile(arr.reshape(-1, 16).T, (8, 1))


def _host_prep(h, src, dst, W_node, b_node, att, w_scale, bias, ln_gamma, ln_beta):
    src = src.astype(np.int64)
    dst = dst.astype(np.int64)

    deg = np.bincount(dst, minlength=N_NODES)

    # per-core window assignment (degree-balanced)
    win_of = np.zeros(N_NODES, np.int32)
    slot_of = np.zeros(N_NODES, np.int32)
    nodeid = np.zeros((NCORES, NWIN, 128), np.int64)
    valid = np.zeros((NCORES, NWIN, 128), bool)
    for k in range(NCORES):
        nodes = np.arange(k * NPC, (k + 1) * NPC)
        order = nodes[np.argsort(-deg[nodes], kind="stable")]
        load = np.zeros(NWIN, np.int64)
        cnt = np.zeros(NWIN, np.int64)
        for n in order:
            cand = np.where(cnt < 128)[0]
            b = cand[np.argmin(load[cand])]
            win_of[n] = b
            slot_of[n] = cnt[b]
            nodeid[k, b, cnt[b]] = n
            valid[k, b, cnt[b]] = True
            load[b] += deg[n]
            cnt[b] += 1

    # split edges per (core, window) into src<SPLIT and src>=SPLIT groups
    core_of_edge = dst // NPC
    win_of_edge = win_of[dst]
    eA = {}
    eB = {}
    maxA = maxB = 0
    for k in range(NCORES):
        em = core_of_edge == k
        for w in range(NWIN):
            sel = em & (win_of_edge == w)
            es = src[sel]
            ed = dst[sel]
            a = es < SPLIT
            eA[(k, w)] = (es[a], ed[a])
            eB[(k, w)] = (es[~a], ed[~a])
            maxA = max(maxA, a.sum())
            maxB = max(maxB, (~a).sum())
    GA = max(1, int(-(-maxA // 128)))
    GB = max(1, int(-(-maxB // 128)))
    T2 = GA + GB

    widx = np.zeros((NCORES, NWIN, 128, T2 * 8), np.int16)
    wrel = np.full((NCORES, NWIN, 128, T2), 255.0, np.float32)
    for k in range(NCORES):
        for w in range(NWIN):
            esA, edA = eA[(k, w)]
            esB, edB = eB[(k, w)]
            widx[k, w, :, 0:GA * 8] = _wrap_idx(esA, GA * 128)
            widx[k, w, :, GA * 8:T2 * 8] = _wrap_idx(esB - SPLIT, GB * 128)
            # edge i -> slot (i%128, group i//128); group g of A -> col g,
            # group g of B -> col GA+g
            ra = np.full(GA * 128, 255.0, np.float32)
            ra[:len(edA)] = slot_of[edA]
            rb = np.full(GB * 128, 255.0, np.float32)
            rb[:len(edB)] = slot_of[edB]
            wrel[k, w, :, 0:GA] = ra.reshape(GA, 128).T
            wrel[k, w, :, GA:T2] = rb.reshape(GB, 128).T

    # transposed one-hots: oht[q, j*128+p] = 1 if wrel[p, j] == q
    qs = np.arange(128, dtype=np.float32)
    ohtb = np.zeros((NCORES, NWIN, 128, T2 * 128), ml_dtypes.bfloat16)
    for k in range(NCORES):
        for w in range(NWIN):
            oh = (wrel[k, w][None, :, :] == qs[:, None, None])  # [q, p, j]
            ohtb[k, w] = oh.transpose(0, 2, 1).reshape(128, T2 * 128).astype(ml_dtypes.bfloat16)

    # weight-derived constants
    Wn3 = W_node.reshape(H, F, F)            # (h, f_out, g)
    att_l = att[:, :F]
    att_r = att[:, F:]
    Ael = np.einsum('hfg,hf->gh', Wn3, att_l).astype(np.float32)
    Aer = np.einsum('hfg,hf->gh', Wn3, att_r).astype(np.float32)
    # row layout: wv 0:384 | er 384:387 | el 387:390 (er first: 128-aligned
    # at subrow 4n+3 for the window-node er gather)
    wcomb = np.concatenate([W_node.T, Aer, Ael], axis=1)          # [128, 390]
    b3 = b_node.reshape(H, F)
    cel = (b3 * att_l).sum(1)
    cer = (b3 * att_r).sum(1)
    browv = np.concatenate([b_node, cer, cel]).astype(np.float32)  # [390]
    brow = np.tile(browv[None, :], (128, 1)).astype(np.float32)
    ident = np.eye(128, dtype=np.float32)
    iotaw = np.tile(np.arange(128, dtype=np.float32)[None, :], (128, T2))
    gam = np.tile(ln_gamma[None, :], (128, 1)).astype(np.float32)
    bet = np.tile(ln_beta[None, :], (128, 1)).astype(np.float32)

    zb = bool(np.all(browv == 0.0))
    gb1 = bool(np.all(ln_gamma == 1.0) and np.all(ln_beta == 0.0))
    bf = ml_dtypes.bfloat16
    common = {
        "hTb": np.ascontiguousarray(h.T).astype(bf),
        "wcomb": wcomb.astype(bf),
        "brow": brow,
        "iotaw": np.ascontiguousarray(iotaw).astype(bf),
        "ident": ident.astype(bf),
        "wsc": np.ascontiguousarray(w_scale).astype(bf),
        "gam": gam, "bet": bet,
    }
    hb = h.astype(np.float32) + bias[None, :].astype(np.float32)
    in_maps = []
    for k in range(NCORES):
        m = dict(common)
        m["widx"] = np.ascontiguousarray(widx[k])
        m["wdb"] = np.ascontiguousarray(wrel[k]).astype(bf)
        m["ohtb"] = np.ascontiguousarray(ohtb[k])
        # subrow indices of the er triple in each own-node row ([N*4,128] view)
        m["wnid"] = np.ascontiguousarray(
            (nodeid[k] * 4 + 3).astype(np.int32).reshape(NWIN, 128, 1))
        m["hperm"] = np.ascontiguousarray(hb[nodeid[k].reshape(-1)])
        in_maps.append(m)
    return GA, GB, zb, gb1, in_maps, nodeid, valid


def kernel(h, src, dst, W_node, b_node, att, w_scale, bias, ln_gamma, ln_beta,
           _want_trace=False):
    GA, GB, zb, gb1, in_maps, nodeid, valid = _host_prep(
        h, src, dst, W_node, b_node, att, w_scale, bias, ln_gamma, ln_beta)
    key = (GA, GB, zb, gb1)
    if key not in _PROGRAM_CACHE:
        _PROGRAM_CACHE[key] = _build_program(GA, GB, zb, gb1)
    nc = _PROGRAM_CACHE[key]
    res = run_bass_kernel_spmd(nc, in_maps, list(range(NCORES)), trace=_want_trace)
    out = np.zeros((N_NODES, F), np.float32)
    for k in range(NCORES):
        rows = res.results[k]["outy"].reshape(NWIN, 128, F)
        v = valid[k]
        out[nodeid[k][v]] = rows[v]
    if _want_trace:
        kernel._last_exec_time_ns = res.exec_time_ns
        kernel._last_trace = res.instructions_and_trace
    return out


# revision 20
# speedup vs baseline: 1.1178x; 1.1178x over previous
"""Multi-head GAT layer on 8 Trainium2 NeuronCores (Bass/Tile).

Strategy: nodes are sharded 6250/core (8 cores). Each core handles all edges
whose dst lands in its shard. Edges are grouped into 49 "windows" of 128
dst-slots (host-balanced by degree).

v3 design:
- node table (wv | el | er) stored bf16 in DRAM as [N, 512] rows (1024B,
  256B-aligned for dma_gather).
- P1 builds the table with 4-tile batched DMAs (loads on Sync, stores on
  Scalar HWDGE queue) and bf16 matmuls.
- P2 fetches wv|el for all edges of a window with TWO dma_gather
  instructions (int16 indices, table split in halves at row S) instead of
  17 per-column indirect DMAs. Per-edge er[dst] is computed on the tensor
  engine: one tiny indirect gather of the window's own er rows plus
  host-uploaded transposed one-hot matrices. Edge softmax weights + one-hot
  scatter matmul in bf16; normalize, project, residual-add per window.
- LayerNorm is split: per-window mean/var (DVE only), a single Sqrt at the
  end (avoids Exp<->Sqrt activation-table thrashing), one big output store.
"""
import sys, os, types, ctypes, contextlib

if '/opt/trn_rl_repo' not in sys.path:
    sys.path.insert(0, '/opt/trn_rl_repo')


def _install_profile_hook():
    try:
        import antenv.axon_hooks  # noqa
        return
    except ImportError:
        pass
    try:
        import antenv
    except ImportError:
        return
    so_path = "/opt/axon/libaxon_pjrt.so"
    hook = None
    if os.path.exists(so_path):
        lib = ctypes.CDLL(so_path)
        if hasattr(lib, "axon_start_nrt_profile"):
            lib.axon_start_nrt_profile.argtypes = [ctypes.POINTER(ctypes.c_int64), ctypes.c_size_t]
            lib.axon_start_nrt_profile.restype = ctypes.c_int64
            lib.axon_stop_nrt_profile.argtypes = [ctypes.c_char_p]
            lib.axon_stop_nrt_profile.restype = ctypes.c_int64

            @contextlib.contextmanager
            def _hook(output_dir, device_ids):
                import jax
                jax.devices()
                if device_ids:
                    ids = (ctypes.c_int64 * len(device_ids))(*device_ids)
                    rc = lib.axon_start_nrt_profile(ids, len(device_ids))
                else:
                    rc = lib.axon_start_nrt_profile(None, 0)
                if rc != 0:
                    raise RuntimeError(f"axon_start_nrt_profile rc={rc}")
                try:
                    yield
                finally:
                    n = lib.axon_stop_nrt_profile(str(output_dir).encode())
                    print(f"ntff profile: {n} file(s) -> {output_dir}", file=sys.stderr)
            hook = _hook
    mod = types.ModuleType("antenv.axon_hooks")
    state = {"hook": hook}
    mod.set_axon_ntff_profile_hook = lambda h: state.__setitem__("hook", h)
    mod.get_axon_ntff_profile_hook = lambda: state["hook"]
    sys.modules["antenv.axon_hooks"] = mod
    antenv.axon_hooks = mod


_install_profile_hook()

import numpy as np
import ml_dtypes
from concourse import bass, bacc, mybir, tile
from concourse.bass_utils import run_bass_kernel_spmd

N_NODES = 50000
F = 128
H = 3
NCORES = 8
NPC = N_NODES // NCORES          # 6250 nodes per core
NWIN = (NPC + 127) // 128        # 49 windows per core
NEG_SLOPE = 0.2
LN_EPS = 1e-5
ROWW = 512                        # table row: wv 384 | el 3 | er 3 | pad
SPLIT = 25088                     # table half split (int16 index range)

f32 = mybir.dt.float32
bf16 = mybir.dt.bfloat16
i32 = mybir.dt.int32
i16 = mybir.dt.int16

_PROGRAM_CACHE = {}


def _build_program(GA, GB, zb, gb1):
    T2 = GA + GB
    nc = bacc.Bacc("TRN2", target_bir_lowering=False, debug=False,
                   enable_asserts=False, num_devices=NCORES,
                   dynamic_dma_scratch_size=65536)
    NT = (N_NODES + 127) // 128   # 391 table tiles (last partial)

    hTb_in = nc.dram_tensor("hTb", [F, N_NODES], bf16, kind="ExternalInput").ap()
    wcomb_in = nc.dram_tensor("wcomb", [128, 390], bf16, kind="ExternalInput").ap()
    brow_in = nc.dram_tensor("brow", [128, 390], f32, kind="ExternalInput").ap()
    hpermT_in = nc.dram_tensor("hpermT", [F, NWIN * 128], bf16, kind="ExternalInput").ap()
    aerb_in = nc.dram_tensor("aerb", [128, 3], bf16, kind="ExternalInput").ap()
    crow_in = nc.dram_tensor("crow", [128, 3], bf16, kind="ExternalInput").ap()
    ident_in = nc.dram_tensor("ident", [128, 128], bf16, kind="ExternalInput").ap()
    wsc_in = nc.dram_tensor("wsc", [H * F, F], bf16, kind="ExternalInput").ap()
    gam_in = nc.dram_tensor("gam", [128, F], f32, kind="ExternalInput").ap()
    bet_in = nc.dram_tensor("bet", [128, F], f32, kind="ExternalInput").ap()
    hperm_in = nc.dram_tensor("hperm", [NWIN * 128, F], f32, kind="ExternalInput").ap()
    widx_in = nc.dram_tensor("widx", [NWIN, 128, T2 * 8], i16, kind="ExternalInput").ap()
    ohb_in = nc.dram_tensor("ohb", [NWIN, 128, T2 * 128], bf16, kind="ExternalInput").ap()
    ohtb_in = nc.dram_tensor("ohtb", [NWIN, 128, T2 * 128], bf16, kind="ExternalInput").ap()

    tableA = nc.dram_tensor("tableA", [SPLIT, ROWW], bf16).ap()
    tableB = nc.dram_tensor("tableB", [N_NODES - SPLIT, ROWW], bf16).ap()
    outy = nc.dram_tensor("outy", [NWIN * 128, F], f32, kind="ExternalOutput").ap()

    inv_f = 1.0 / F

    with tile.TileContext(nc) as tc:
        with tc.tile_pool(name="const", bufs=1) as cpool:
            wcomb = cpool.tile([128, 390], bf16)
            nc.sync.dma_start(wcomb[:], wcomb_in[:])
            brow = cpool.tile([128, 390], f32)
            nc.sync.dma_start(brow[:], brow_in[:])
            hpermT = cpool.tile([128, NWIN * 128], bf16, tag="hpermT")
            nc.sync.dma_start(hpermT[:], hpermT_in[:])
            aerb = cpool.tile([128, 3], bf16, tag="aerb")
            nc.sync.dma_start(aerb[:], aerb_in[:])
            crow = cpool.tile([128, 3], bf16, tag="crow")
            nc.sync.dma_start(crow[:], crow_in[:])
            ident = cpool.tile([128, 128], bf16)
            nc.sync.dma_start(ident[:], ident_in[:])
            gam = cpool.tile([128, F], f32)
            nc.sync.dma_start(gam[:], gam_in[:])
            bet = cpool.tile([128, F], f32)
            nc.sync.dma_start(bet[:], bet_in[:])
            wsc_c = []
            for c in range(H):
                t = cpool.tile([128, F], bf16, tag=f"wsc{c}")
                nc.sync.dma_start(t[:], wsc_in[c * 128:(c + 1) * 128, :])
                wsc_c.append(t)
            # persistent LN state
            xc_all = cpool.tile([128, NWIN * F], f32, tag="xc_all")
            vp_all = cpool.tile([128, NWIN], f32, tag="vp_all")

            # ---- P1: build wv/el/er table (4-tile batches)
            with (
                tc.tile_pool(name="p1l", bufs=3) as lpool,
                tc.tile_pool(name="p1d", bufs=3) as dpool,
                tc.tile_pool(name="p1p", bufs=2, space="PSUM") as p1ps,
            ):
                NB = 97  # full 4-tile batches
                for b in range(NB):
                    n0 = b * 512
                    ht4 = lpool.tile([128, 512], bf16, tag="ht")
                    nc.sync.dma_start(ht4[:], hTb_in[:, n0:n0 + 512])
                    sb4 = dpool.tile([128, 4, 390], bf16, tag="sb")
                    for t in range(4):
                        ps = p1ps.tile([128, 390], f32, tag=f"p{t}")
                        nc.tensor.matmul(out=ps[:], lhsT=ht4[:, t * 128:(t + 1) * 128],
                                         rhs=wcomb[:], start=True, stop=True)
                        if zb:
                            if t != 3:
                                nc.vector.tensor_copy(sb4[:, t, :], ps[:])
                            else:
                                nc.scalar.copy(sb4[:, t, :], ps[:])
                        else:
                            nc.vector.tensor_tensor(out=sb4[:, t, :], in0=ps[:], in1=brow[:],
                                                    op=mybir.AluOpType.add)
                    if n0 < SPLIT:
                        dst = tableA[n0:n0 + 512, 0:390]
                    else:
                        dst = tableB[n0 - SPLIT:n0 - SPLIT + 512, 0:390]
                    nc.scalar.dma_start(dst.rearrange("(t p) c -> p t c", p=128),
                                        sb4[:])
                for tt in range(4 * NB, NT):
                    n0 = tt * 128
                    n1 = min(n0 + 128, N_NODES)
                    p = n1 - n0
                    ht1 = lpool.tile([128, 128], bf16, tag="ht1")
                    nc.sync.dma_start(ht1[:, :p], hTb_in[:, n0:n1])
                    ps = p1ps.tile([128, 390], f32, tag="p0")
                    nc.tensor.matmul(out=ps[:p, :], lhsT=ht1[:, :p], rhs=wcomb[:],
                                     start=True, stop=True)
                    sb1 = dpool.tile([128, 390], bf16, tag="sb1")
                    if zb:
                        nc.vector.tensor_copy(sb1[:p, :], ps[:p, :])
                    else:
                        nc.vector.tensor_tensor(out=sb1[:p, :], in0=ps[:p, :], in1=brow[:p, :],
                                                op=mybir.AluOpType.add)
                    nc.scalar.dma_start(tableB[n0 - SPLIT:n1 - SPLIT, 0:390],
                                        sb1[:p, :])

            tabA = tableA[:]
            tabB = tableB[:]

            # ---- P2: edge windows
            with (
                tc.tile_pool(name="gath", bufs=3) as gpool,
                tc.tile_pool(name="ohp", bufs=2) as opool,
                tc.tile_pool(name="ohtp", bufs=2) as otpool,
                tc.tile_pool(name="small", bufs=3) as spool,
                tc.tile_pool(name="edge", bufs=3) as epool,
                tc.tile_pool(name="fin", bufs=3) as fpool,
                tc.tile_pool(name="psw", bufs=2, space="PSUM") as pswin,
                tc.tile_pool(name="pse", bufs=2, space="PSUM") as pserr,
                tc.tile_pool(name="pst", bufs=2, space="PSUM") as pstab,
                tc.tile_pool(name="psx", bufs=2, space="PSUM") as psaux,
            ):
                for w in range(NWIN):
                    widx = spool.tile([128, T2 * 8], i16, tag="wx")
                    nc.sync.dma_start(widx[:], widx_in[w])
                    oht = otpool.tile([128, T2 * 128], bf16, tag="oht")
                    nc.sync.dma_start(oht[:], ohtb_in[w])
                    oh = opool.tile([128, T2, 128], bf16, tag="oh")
                    nc.sync.dma_start(oh[:], ohb_in[w])
                    hpw = spool.tile([128, F], f32, tag="hp")
                    nc.scalar.dma_start(hpw[:], hperm_in[w * 128:(w + 1) * 128, :])
                    # own-node er via matmul: er_win = hperm_w @ Aer (+ cer)
                    erps = pserr.tile([128, T2 * 3 + 3], f32, tag="erp")
                    nc.tensor.matmul(out=erps[:, T2 * 3:T2 * 3 + 3],
                                     lhsT=hpermT[:, w * 128:(w + 1) * 128],
                                     rhs=aerb[:], start=True, stop=True)
                    erw = spool.tile([128, 3], bf16, tag="erw")
                    if zb:
                        nc.vector.tensor_copy(erw[:], erps[:, T2 * 3:T2 * 3 + 3])
                    else:
                        nc.vector.tensor_tensor(out=erw[:], in0=erps[:, T2 * 3:T2 * 3 + 3],
                                                in1=crow[:], op=mybir.AluOpType.add)

                    gv = gpool.tile([128, T2, ROWW], bf16, tag="gv")
                    # dma_gather is limited to 1024 indices (8 groups) per call
                    for g0 in range(0, GA, 8):
                        g1 = min(g0 + 8, GA)
                        nc.gpsimd.dma_gather(gv[:, g0:g1, :], tabA,
                                             widx[:, g0 * 8:g1 * 8],
                                             (g1 - g0) * 128, (g1 - g0) * 128, ROWW)
                    for g0 in range(0, GB, 8):
                        g1 = min(g0 + 8, GB)
                        nc.gpsimd.dma_gather(gv[:, GA + g0:GA + g1, :], tabB,
                                             widx[:, (GA + g0) * 8:(GA + g1) * 8],
                                             (g1 - g0) * 128, (g1 - g0) * 128, ROWW)
                    # per-edge er[dst] via transposed one-hots
                    for j in range(T2):
                        nc.tensor.matmul(out=erps[:, j * 3:(j + 1) * 3],
                                         lhsT=oht[:, j * 128:(j + 1) * 128],
                                         rhs=erw[:], start=True, stop=True)
                    erb = epool.tile([128, T2 * 3], bf16, tag="erb")
                    nc.vector.tensor_copy(erb[:], erps[:, 0:T2 * 3])

                    nk = T2 * 3
                    attn = epool.tile([128, T2, 3], f32, tag="attn")
                    nc.vector.tensor_tensor(out=attn[:], in0=gv[:, :, 387:390],
                                            in1=erb[:].rearrange("p (a b) -> p a b", b=3),
                                            op=mybir.AluOpType.add)
                    at2 = epool.tile([128, nk], f32, tag="at2")
                    nc.vector.tensor_scalar_mul(at2[:], attn[:].rearrange("p a b -> p (a b)"),
                                                NEG_SLOPE)
                    at3 = epool.tile([128, nk], f32, tag="at3")
                    nc.vector.tensor_tensor(out=at3[:], in0=attn[:].rearrange("p a b -> p (a b)"),
                                            in1=at2[:], op=mybir.AluOpType.max)
                    ew = epool.tile([128, T2, 3], bf16, tag="ew")
                    nc.scalar.activation(ew[:].rearrange("p a b -> p (a b)"), at3[:],
                                         mybir.ActivationFunctionType.Exp)

                    # scale gathered wv rows by edge weights in place
                    for c in range(H):
                        nc.vector.tensor_tensor(
                            out=gv[:, :, c * 128:(c + 1) * 128],
                            in0=gv[:, :, c * 128:(c + 1) * 128],
                            in1=ew[:, :, c:c + 1].to_broadcast([128, T2, 128]),
                            op=mybir.AluOpType.mult)

                    ps = pswin.tile([128, 387], f32, tag="win")
                    for j in range(T2):
                        nc.tensor.matmul(out=ps[:, 0:384], lhsT=oh[:, j, :],
                                         rhs=gv[:, j, 0:384],
                                         start=(j == 0), stop=(j == T2 - 1))
                    # separate pass: two open accumulation groups must not
                    # interleave within one PSUM bank
                    for j in range(T2):
                        nc.tensor.matmul(out=ps[:, 384:387], lhsT=oh[:, j, :],
                                         rhs=ew[:, j, :],
                                         start=(j == 0), stop=(j == T2 - 1))

                    dmx = spool.tile([128, 3], f32, tag="dmx")
                    nc.vector.tensor_scalar(out=dmx[:], in0=ps[:, 384:387], scalar1=1e-9,
                                            scalar2=None, op0=mybir.AluOpType.max)
                    dr = spool.tile([128, 3], f32, tag="dr")
                    nc.vector.reciprocal(dr[:], dmx[:])
                    agg = epool.tile([128, 3, 128], bf16, tag="agg")
                    nc.vector.tensor_tensor(
                        out=agg[:],
                        in0=ps[:, 0:384].rearrange("p (c f) -> p c f", f=128),
                        in1=dr[:].rearrange("p (c o) -> p c o", o=1).to_broadcast([128, 3, 128]),
                        op=mybir.AluOpType.mult)

                    pp = psaux.tile([128, F], f32, tag="proj")
                    for c in range(H):
                        tp = pstab.tile([128, 128], bf16, tag="tp")
                        nc.tensor.transpose(out=tp[:], in_=agg[:, c, :], identity=ident[:])
                        aggT = epool.tile([128, 128], bf16, tag="aggT")
                        nc.vector.tensor_copy(aggT[:], tp[:])
                        nc.tensor.matmul(out=pp[:], lhsT=aggT[:], rhs=wsc_c[c][:],
                                         start=(c == 0), stop=(c == H - 1))

                    # residual + LN stats (mean/var); normalization deferred
                    xw = fpool.tile([128, F], f32, tag="xw")
                    sm = fpool.tile([128, 1], f32, tag="sm")
                    nc.vector.tensor_tensor(out=xw[:], in0=pp[:], in1=hpw[:],
                                            op=mybir.AluOpType.add)
                    nc.vector.reduce_sum(sm[:], xw[:], axis=mybir.AxisListType.X)
                    mu = fpool.tile([128, 1], f32, tag="mu")
                    nc.vector.tensor_scalar_mul(mu[:], sm[:], inv_f)
                    nc.vector.tensor_scalar(out=xc_all[:, w * F:(w + 1) * F], in0=xw[:],
                                            scalar1=mu[:, 0:1], scalar2=None,
                                            op0=mybir.AluOpType.subtract)
                    junk = fpool.tile([128, F], f32, tag="junk")
                    nc.vector.tensor_tensor(out=junk[:], in0=xc_all[:, w * F:(w + 1) * F],
                                            in1=xc_all[:, w * F:(w + 1) * F],
                                            op=mybir.AluOpType.mult)
                    vs = fpool.tile([128, 1], f32, tag="vs")
                    nc.vector.reduce_sum(vs[:], junk[:], axis=mybir.AxisListType.X)
                    nc.vector.tensor_scalar_mul(vp_all[:, w:w + 1], vs[:], inv_f)

                # ---- P3: single sqrt, normalize, relu, one big store
                vpe = cpool.tile([128, NWIN], f32, tag="vpe")
                nc.vector.tensor_scalar(out=vpe[:], in0=vp_all[:], scalar1=LN_EPS,
                                        scalar2=None, op0=mybir.AluOpType.add)
                rv = cpool.tile([128, NWIN], f32, tag="rv")
                nc.vector.reciprocal(rv[:], vpe[:])
                rstd = cpool.tile([128, NWIN], f32, tag="rstd")
                nc.scalar.activation(rstd[:], rv[:], mybir.ActivationFunctionType.Sqrt)
                for w in range(NWIN):
                    y1 = fpool.tile([128, F], f32, tag="y1")
                    if gb1:
                        # fuse on the scalar engine: y = Relu(xc * rstd)
                        nc.scalar.activation(xc_all[:, w * F:(w + 1) * F],
                                             xc_all[:, w * F:(w + 1) * F],
                                             mybir.ActivationFunctionType.Relu,
                                             scale=rstd[:, w:w + 1])
                    else:
                        nc.vector.tensor_scalar_mul(y1[:], xc_all[:, w * F:(w + 1) * F],
                                                    rstd[:, w:w + 1])
                        y2 = fpool.tile([128, F], f32, tag="y2")
                        nc.vector.tensor_tensor(out=y2[:], in0=y1[:], in1=gam[:],
                                                op=mybir.AluOpType.mult)
                        y3 = fpool.tile([128, F], f32, tag="y3")
                        nc.vector.tensor_tensor(out=y3[:], in0=y2[:], in1=bet[:],
                                                op=mybir.AluOpType.add)
                        nc.vector.tensor_scalar(out=xc_all[:, w * F:(w + 1) * F], in0=y3[:],
                                                scalar1=0.0, scalar2=None,
                                                op0=mybir.AluOpType.max)
                nc.scalar.dma_start(outy.rearrange("(w p) f -> p w f", p=128),
                                    xc_all[:].rearrange("p (w f) -> p w f", f=F))

    nc.compile()
    return nc


def _wrap_idx(ix, cap):
    """Pad index list to cap and wrap for dma_gather: idx i -> [p, s] with
    i = s*16 + p%16, replicated across the 8 16-partition groups."""
    arr = np.zeros(cap, np.int16)
    arr[:len(ix)] = ix
    return np.tile(arr.reshape(-1, 16).T, (8, 1))


def _host_prep(h, src, dst, W_node, b_node, att, w_scale, bias, ln_gamma, ln_beta):
    src = src.astype(np.int64)
    dst = dst.astype(np.int64)

    deg = np.bincount(dst, minlength=N_NODES)

    # per-core window assignment (degree-balanced)
    win_of = np.zeros(N_NODES, np.int32)
    slot_of = np.zeros(N_NODES, np.int32)
    nodeid = np.zeros((NCORES, NWIN, 128), np.int64)
    valid = np.zeros((NCORES, NWIN, 128), bool)
    for k in range(NCORES):
        nodes = np.arange(k * NPC, (k + 1) * NPC)
        order = nodes[np.argsort(-deg[nodes], kind="stable")]
        load = np.zeros(NWIN, np.int64)
        cnt = np.zeros(NWIN, np.int64)
        for n in order:
            cand = np.where(cnt < 128)[0]
            b = cand[np.argmin(load[cand])]
            win_of[n] = b
            slot_of[n] = cnt[b]
            nodeid[k, b, cnt[b]] = n
            valid[k, b, cnt[b]] = True
            load[b] += deg[n]
            cnt[b] += 1

    # split edges per (core, window) into src<SPLIT and src>=SPLIT groups
    core_of_edge = dst // NPC
    win_of_edge = win_of[dst]
    eA = {}
    eB = {}
    maxA = maxB = 0
    for k in range(NCORES):
        em = core_of_edge == k
        for w in range(NWIN):
            sel = em & (win_of_edge == w)
            es = src[sel]
            ed = dst[sel]
            a = es < SPLIT
            eA[(k, w)] = (es[a], ed[a])
            eB[(k, w)] = (es[~a], ed[~a])
            maxA = max(maxA, a.sum())
            maxB = max(maxB, (~a).sum())
    GA = max(1, int(-(-maxA // 128)))
    GB = max(1, int(-(-maxB // 128)))
    T2 = GA + GB

    widx = np.zeros((NCORES, NWIN, 128, T2 * 8), np.int16)
    wrel = np.full((NCORES, NWIN, 128, T2), 255.0, np.float32)
    for k in range(NCORES):
        for w in range(NWIN):
            esA, edA = eA[(k, w)]
            esB, edB = eB[(k, w)]
            widx[k, w, :, 0:GA * 8] = _wrap_idx(esA, GA * 128)
            widx[k, w, :, GA * 8:T2 * 8] = _wrap_idx(esB - SPLIT, GB * 128)
            # edge i -> slot (i%128, group i//128); group g of A -> col g,
            # group g of B -> col GA+g
            ra = np.full(GA * 128, 255.0, np.float32)
            ra[:len(edA)] = slot_of[edA]
            rb = np.full(GB * 128, 255.0, np.float32)
            rb[:len(edB)] = slot_of[edB]
            wrel[k, w, :, 0:GA] = ra.reshape(GA, 128).T
            wrel[k, w, :, GA:T2] = rb.reshape(GB, 128).T

    # one-hots: ohb[p, j*128+q] = 1 if wrel[p, j] == q (scatter lhsT), and
    # transposed oht[q, j*128+p] (er-gather lhsT)
    qs = np.arange(128, dtype=np.float32)
    ohb = np.zeros((NCORES, NWIN, 128, T2 * 128), ml_dtypes.bfloat16)
    ohtb = np.zeros((NCORES, NWIN, 128, T2 * 128), ml_dtypes.bfloat16)
    for k in range(NCORES):
        for w in range(NWIN):
            oh = (wrel[k, w][None, :, :] == qs[:, None, None])  # [q, p, j]
            ohb[k, w] = oh.transpose(1, 2, 0).reshape(128, T2 * 128).astype(ml_dtypes.bfloat16)
            ohtb[k, w] = oh.transpose(0, 2, 1).reshape(128, T2 * 128).astype(ml_dtypes.bfloat16)

    # weight-derived constants
    Wn3 = W_node.reshape(H, F, F)            # (h, f_out, g)
    att_l = att[:, :F]
    att_r = att[:, F:]
    Ael = np.einsum('hfg,hf->gh', Wn3, att_l).astype(np.float32)
    Aer = np.einsum('hfg,hf->gh', Wn3, att_r).astype(np.float32)
    # row layout: wv 0:384 | er 384:387 | el 387:390 (er first: 128-aligned
    # at subrow 4n+3 for the window-node er gather)
    wcomb = np.concatenate([W_node.T, Aer, Ael], axis=1)          # [128, 390]
    b3 = b_node.reshape(H, F)
    cel = (b3 * att_l).sum(1)
    cer = (b3 * att_r).sum(1)
    browv = np.concatenate([b_node, cer, cel]).astype(np.float32)  # [390]
    brow = np.tile(browv[None, :], (128, 1)).astype(np.float32)
    ident = np.eye(128, dtype=np.float32)
    gam = np.tile(ln_gamma[None, :], (128, 1)).astype(np.float32)
    bet = np.tile(ln_beta[None, :], (128, 1)).astype(np.float32)

    zb = bool(np.all(browv == 0.0))
    gb1 = bool(np.all(ln_gamma == 1.0) and np.all(ln_beta == 0.0))
    bf = ml_dtypes.bfloat16
    common = {
        "hTb": np.ascontiguousarray(h.T).astype(bf),
        "wcomb": wcomb.astype(bf),
        "brow": brow,
        "ident": ident.astype(bf),
        "wsc": np.ascontiguousarray(w_scale).astype(bf),
        "gam": gam, "bet": bet,
        "aerb": np.ascontiguousarray(Aer).astype(bf),
        "crow": np.tile(cer[None, :], (128, 1)).astype(bf),
    }
    hb = h.astype(np.float32) + bias[None, :].astype(np.float32)
    in_maps = []
    for k in range(NCORES):
        m = dict(common)
        m["widx"] = np.ascontiguousarray(widx[k])
        m["ohb"] = np.ascontiguousarray(ohb[k])
        m["ohtb"] = np.ascontiguousarray(ohtb[k])
        m["hperm"] = np.ascontiguousarray(hb[nodeid[k].reshape(-1)])
        m["hpermT"] = np.ascontiguousarray(
            h.astype(np.float32)[nodeid[k].reshape(-1)].T).astype(bf)
        in_maps.append(m)
    return GA, GB, zb, gb1, in_maps, nodeid, valid


def kernel(h, src, dst, W_node, b_node, att, w_scale, bias, ln_gamma, ln_beta,
           _want_trace=False):
    GA, GB, zb, gb1, in_maps, nodeid, valid = _host_prep(
        h, src, dst, W_node, b_node, att, w_scale, bias, ln_gamma, ln_beta)
    key = (GA, GB, zb, gb1)
    if key not in _PROGRAM_CACHE:
        _PROGRAM_CACHE[key] = _build_program(GA, GB, zb, gb1)
    nc = _PROGRAM_CACHE[key]
    res = run_bass_kernel_spmd(nc, in_maps, list(range(NCORES)), trace=_want_trace)
    out = np.zeros((N_NODES, F), np.float32)
    for k in range(NCORES):
        rows = res.results[k]["outy"].reshape(NWIN, 128, F)
        v = valid[k]
        out[nodeid[k][v]] = rows[v]
    if _want_trace:
        kernel._last_exec_time_ns = res.exec_time_ns
        kernel._last_trace = res.instructions_and_trace
    return out
